# revision 30
# baseline (speedup 1.0000x reference)
"""Trainium2 Bass kernel for GQA attention (B=2, S=2048, HID=2048, 16 q-heads,
4 kv-heads, HD=128, RoPE, softmax, output projection).

Sharding: 8 cores = (2 batches) x (4 query-head groups of 4 heads). Each core
owns one batch's hidden states, 4 query heads, and the single kv head those
query heads attend to (GQA group), plus the matching 512-row slice of Wo.
Core (b, g) computes a [S, HID] partial of the output projection; the host
sums the 4 partials per batch (row-scaled softmax commutes with the
projection, so normalization happens on-device per head).

Dataflow per core (all layouts transposed so the contraction dim sits on
SBUF partitions; no DMA transposes needed for fp32):
  x^T via PE transposes -> Q^T/K^T/V^T projections (fp32r matmuls)
  -> RoPE on Q^T/K^T (DVE, sign-folded sin table)
  -> scores^T tiles = K^T_chunk.T @ Q^T (fp32r) -> exp on ACT (bf16 E tiles)
  -> softmax denominators via ones-vector matmul (PE), reciprocal (DVE),
     rank-1 ones x recip broadcast matmul (PE)
  -> attn@V with PE-transposed V chunks (bf16) -> normalize (DVE)
  -> output projection vs Wo slice (fp32r) -> partial [S, HID] to HBM.
"""

import sys
import types

sys.path.insert(0, "/opt/trn_rl_repo")

import numpy as np

B, S, HID = 2, 2048, 2048
NH, NKV, HD = 16, 4, 128
GROUPS = NH // NKV          # q heads per kv head == heads per core
ROPE_THETA = 10000.0
P = 128                     # SBUF partitions
SB = 512                    # s-block (matmul moving dim / psum bank)
N_CORES = 8

_built = None


def _install_ntff_hook():
    """antenv.axon_hooks is missing from the agent image, which silently
    disables trace=True; recreate it and register the ctypes NTFF hook."""
    if "antenv.axon_hooks" in sys.modules:
        return
    m = types.ModuleType("antenv.axon_hooks")
    m._hook = None
    m.set_axon_ntff_profile_hook = lambda h: setattr(m, "_hook", h)
    m.get_axon_ntff_profile_hook = lambda: m._hook
    sys.modules["antenv.axon_hooks"] = m
    try:
        import antenv

        antenv.axon_hooks = m
    except ImportError:
        pass
    try:
        sys.path.insert(0, "/root/.axon_site/trn_agent_boot")
        from trn_boot import _ntff_profile_via_ctypes

        hook = _ntff_profile_via_ctypes("/opt/axon/libaxon_pjrt.so")
        if hook is not None:
            m.set_axon_ntff_profile_hook(hook)
    except Exception:
        pass


_install_ntff_hook()


def rope_tables():
    """cos table and sign-folded sin table in [HD, S] (transposed) layout.

    sin_signed[d] = -sin for d < HD/2, +sin for d >= HD/2, so RoPE becomes
    out = q * cos + shifted(q) * sin_signed with shifted(q) a partition-half
    swap.
    """
    half = HD // 2
    inv_freq = 1.0 / (ROPE_THETA ** (np.arange(0, HD, 2, dtype=np.float64) / HD))
    t = np.arange(S, dtype=np.float64)
    freqs = np.outer(t, inv_freq)                      # [S, 64]
    emb = np.concatenate([freqs, freqs], axis=-1)      # [S, 128]
    cos_t = np.cos(emb).T.astype(np.float32).copy()    # [128, S]
    sin = np.sin(emb).T.astype(np.float32)
    sin_signed = sin.copy()
    sin_signed[:half] *= -1.0
    return cos_t, np.ascontiguousarray(sin_signed)


def build_bass(s=S, hid=HID):
    import concourse.mybir as mybir
    from concourse import bacc
    from concourse.tile import TileContext

    F32 = mybir.dt.float32
    F32R = mybir.dt.float32r
    BF16 = mybir.dt.bfloat16
    EXP = mybir.ActivationFunctionType.Exp
    MUL = mybir.AluOpType.mult

    kc_n = hid // P            # hid contraction chunks
    sb_n = s // SB             # s blocks
    jc_n = s // P              # key chunks
    nh = GROUPS                # heads on this core
    scale = 1.0 / float(np.sqrt(HD))

    nc = bacc.Bacc("TRN2")

    xT_d = nc.dram_tensor("xT", [hid, s], F32R, kind="ExternalInput")
    wq = nc.dram_tensor("wq", [hid, nh * HD], F32R, kind="ExternalInput")
    wk = nc.dram_tensor("wk", [hid, HD], F32R, kind="ExternalInput")
    wv = nc.dram_tensor("wv", [hid, HD], F32R, kind="ExternalInput")
    wo = nc.dram_tensor("wo", [nh * HD, hid], F32, kind="ExternalInput")
    cos_d = nc.dram_tensor("cos_t", [P, s], F32, kind="ExternalInput")
    sin_d = nc.dram_tensor("sin_t", [P, s], F32, kind="ExternalInput")
    ident_d = nc.dram_tensor("ident", [P, P], F32, kind="ExternalInput")
    ones_mat_d = nc.dram_tensor("ones_mat", [P, P], F32, kind="ExternalInput")
    out = nc.dram_tensor("out", [s, hid], F32, kind="ExternalOutput")

    xT_r = xT_d.rearrange("(ko ki) s -> ki ko s", ki=P)
    wq_r = wq.rearrange("(ko ki) m -> ki ko m", ki=P)
    wk_r = wk.rearrange("(ko ki) m -> ki ko m", ki=P)
    wv_r = wv.rearrange("(ko ki) m -> ki ko m", ki=P)
    wo_r = wo.rearrange("(h ki) o -> ki h o", ki=P)

    with TileContext(nc) as tc:
        with (
            tc.tile_pool(name="const", bufs=1) as cpool,
            tc.tile_pool(name="qkvt", bufs=1) as qkvt,
            tc.tile_pool(name="ropetmp", bufs=2) as rtmp,
            tc.tile_pool(name="vtmp", bufs=2) as vtmpp,
        ):
            ident = cpool.tile([P, P], F32, tag="ident")
            nc.scalar.dma_start(ident[:], ident_d[:, :])
            ones_mat = cpool.tile([P, P], F32, tag="onesm")
            nc.scalar.dma_start(ones_mat[:], ones_mat_d[:, :])
            ident_bf = cpool.tile([P, P], BF16, tag="identbf")
            nc.vector.tensor_copy(ident_bf[:], ident[:])
            ones_mat_bf = cpool.tile([P, P], BF16, tag="onesmbf")
            nc.vector.tensor_copy(ones_mat_bf[:], ones_mat[:])

            qT = qkvt.tile([P, nh, s], F32R, tag="qT")       # [d, h, s]
            kT = qkvt.tile([P, s], F32R, tag="kT")           # [d, s]
            vnat = qkvt.tile([P, jc_n, HD], BF16, tag="vn")  # [j, jc, d]

            def rope(dst, src_ps, gs):
                # dst = src * cos + shifted_halves(src) * sin_signed
                h2 = HD // 2
                tmp = rtmp.tile([P, SB], F32, tag="ropetmp")
                nc.vector.tensor_tensor(dst, src_ps[:], cos_t[:, gs : gs + SB], MUL)
                nc.vector.tensor_tensor(
                    tmp[0:h2, :], src_ps[h2:P, :], sin_t[0:h2, gs : gs + SB], MUL
                )
                nc.vector.tensor_tensor(
                    tmp[h2:P, :], src_ps[0:h2, :], sin_t[h2:P, gs : gs + SB], MUL
                )
                nc.vector.tensor_add(dst, dst, tmp[:])

            # ---- Phase 1: x^T, projections, RoPE, V transpose (per s-block) --
            with (
                tc.tile_pool(name="xt", bufs=2) as xtp,
                tc.tile_pool(name="wsb", bufs=1) as wsb,
                tc.tile_pool(name="psA", bufs=2, space="PSUM") as psA,
                tc.tile_pool(name="psB", bufs=2, space="PSUM") as psB,
            ):
                wq_sb = wsb.tile([P, kc_n, nh * HD], F32R, tag="wq")
                wk_sb = wsb.tile([P, kc_n, HD], F32R, tag="wk")
                wv_sb = wsb.tile([P, kc_n, HD], F32R, tag="wv")
                kq = max(1, kc_n // 8)
                for c0 in range(0, kc_n, kq):
                    c1 = min(c0 + kq, kc_n)
                    nc.scalar.dma_start(wq_sb[:, c0:c1, :], wq_r[:, c0:c1, :])
                nc.scalar.dma_start(wk_sb[:], wk_r[:, :, :])
                nc.scalar.dma_start(wv_sb[:], wv_r[:, :, :])
                cos_t = cpool.tile([P, s], F32, tag="cos")
                nc.scalar.dma_start(cos_t[:], cos_d[:, :])
                sin_t = cpool.tile([P, s], F32, tag="sin")
                nc.scalar.dma_start(sin_t[:], sin_d[:, :])

                for sb in range(sb_n):
                    gs = sb * SB
                    xt = xtp.tile([P, kc_n, SB], F32R, tag="xt")
                    kq = max(1, kc_n // 8)
                    for ci, c0 in enumerate(range(0, kc_n, kq)):
                        c1 = min(c0 + kq, kc_n)
                        eng = nc.sync
                        eng.dma_start(
                            xt[:, c0:c1, :], xT_r[:, c0:c1, gs : gs + SB]
                        )

                    for h in range(nh):
                        q_ps = psB.tile([P, SB], F32, tag="proj")
                        for kc in range(kc_n):
                            nc.tensor.matmul(
                                q_ps[:],
                                wq_sb[:, kc, h * HD : (h + 1) * HD],
                                xt[:, kc, :],
                                start=(kc == 0),
                                stop=(kc == kc_n - 1),
                            )
                        rope(qT[:, h, gs : gs + SB], q_ps, gs)

                    k_ps = psB.tile([P, SB], F32, tag="proj")
                    for kc in range(kc_n):
                        nc.tensor.matmul(
                            k_ps[:],
                            wk_sb[:, kc, :],
                            xt[:, kc, :],
                            start=(kc == 0),
                            stop=(kc == kc_n - 1),
                        )
                    rope(kT[:, gs : gs + SB], k_ps, gs)

                    v_ps = psB.tile([P, SB], F32, tag="proj")
                    for kc in range(kc_n):
                        nc.tensor.matmul(
                            v_ps[:],
                            wv_sb[:, kc, :],
                            xt[:, kc, :],
                            start=(kc == 0),
                            stop=(kc == kc_n - 1),
                        )
                    vtmp = vtmpp.tile([P, SB], BF16, tag="vtmp")
                    nc.scalar.copy(vtmp[:], v_ps[:])
                    tps2 = psA.tile([P, SB], BF16, tag="tpsbf")
                    for t in range(SB // P):
                        nc.tensor.transpose(
                            tps2[:, t * P : (t + 1) * P],
                            vtmp[:, t * P : (t + 1) * P],
                            ident_bf[:],
                        )
                    jc0 = gs // P
                    nc.scalar.copy(
                        vnat[:, jc0 : jc0 + SB // P, :],
                        tps2[:].rearrange("p (a b) -> p a b", a=SB // P),
                    )

            # ---- Phase 2: attention + output projection (per i-block) -------
            with (
                tc.tile_pool(name="wo", bufs=1) as wop,
                tc.tile_pool(name="epool", bufs=2) as epool,
                tc.tile_pool(name="osb", bufs=2) as osbp,
                tc.tile_pool(name="recip", bufs=2) as rpool,
                tc.tile_pool(name="outsb", bufs=3) as outp,
                tc.tile_pool(name="psS", bufs=2, space="PSUM") as psS,
                tc.tile_pool(name="psP", bufs=2, space="PSUM") as psP,
                tc.tile_pool(name="psDen", bufs=1, space="PSUM") as psDen,
                tc.tile_pool(name="psO", bufs=1, space="PSUM") as psO,
            ):
                wo_sb = wop.tile([P, nh, hid], F32, tag="wo")
                nc.scalar.dma_start(wo_sb[:], wo_r[:, :, :])
                wo_bf = wop.tile([P, nh, hid], BF16, tag="wobf")
                for wh in range(nh):
                    nc.scalar.copy(wo_bf[:, wh, :], wo_sb[:, wh, :])

                def emit_outproj(po_gi, po_osb):
                    for po_ic in range(SB // P):
                        for po_oc in range(hid // SB):
                            po_ps = psP.tile([P, SB], F32, tag="projps")
                            for po_h in range(nh):
                                nc.tensor.matmul(
                                    po_ps[:],
                                    po_osb[:, po_h, po_ic * P : (po_ic + 1) * P],
                                    wo_bf[:, po_h, po_oc * SB : (po_oc + 1) * SB],
                                    start=(po_h == 0),
                                    stop=(po_h == nh - 1),
                                )
                            po_out = outp.tile([P, SB], F32, tag="outsb")
                            nc.vector.tensor_copy(po_out[:], po_ps[:])
                            nc.sync.dma_start(
                                out[po_gi + po_ic * P : po_gi + (po_ic + 1) * P,
                                    po_oc * SB : (po_oc + 1) * SB],
                                po_out[:],
                            )

                pending = [None]
                for ib in range(sb_n):
                    gi = ib * SB
                    osb = osbp.tile([P, nh, SB], BF16, tag="osb")
                    for h in range(nh):
                        e_t = epool.tile([P, jc_n, SB], BF16, tag="E")
                        den_ps = psDen.tile([P, SB], F32, tag="den")
                        o_ps = psO.tile([P, SB], F32, tag="opsum")
                        esum = rpool.tile([P, SB], BF16, tag="esum")
                        for jp in range(jc_n // 2):
                            jc0, jc1 = 2 * jp, 2 * jp + 1
                            s_ps = psS.tile([P, 2, SB], F32, tag="spsum")
                            nc.tensor.matmul(
                                s_ps[:, 0, :],
                                kT[:, jc0 * P : (jc0 + 1) * P],
                                qT[:, h, gi : gi + SB],
                                start=True,
                                stop=True,
                            )
                            nc.tensor.matmul(
                                s_ps[:, 1, :],
                                kT[:, jc1 * P : (jc1 + 1) * P],
                                qT[:, h, gi : gi + SB],
                                start=True,
                                stop=True,
                            )
                            nc.scalar.activation(
                                e_t[:, jc0 : jc0 + 2, :], s_ps[:], EXP, scale=scale
                            )
                            if jp == 0:
                                nc.vector.tensor_add(
                                    esum[:], e_t[:, 0, :], e_t[:, 1, :]
                                )
                            else:
                                nc.vector.tensor_add(
                                    esum[:], esum[:], e_t[:, jc0, :]
                                )
                                nc.vector.tensor_add(
                                    esum[:], esum[:], e_t[:, jc1, :]
                                )
                            nc.tensor.matmul(
                                o_ps[:],
                                vnat[:, jc0, :],
                                e_t[:, jc0, :],
                                start=(jc0 == 0),
                                stop=False,
                            )
                            nc.tensor.matmul(
                                o_ps[:],
                                vnat[:, jc1, :],
                                e_t[:, jc1, :],
                                start=False,
                                stop=(jc1 == jc_n - 1),
                            )
                        nc.tensor.matmul(
                            den_ps[:],
                            ones_mat_bf[:],
                            esum[:],
                            start=True,
                            stop=True,
                        )
                        recip_sb = rpool.tile([P, SB], F32, tag="recipsb")
                        scratch = rpool.tile([P, SB], F32, tag="rscratch")
                        nc.vector.reciprocal_approx_accurate(
                            recip_sb[:], den_ps[:], scratch[:]
                        )
                        nc.vector.tensor_tensor(
                            osb[:, h, :], o_ps[:], recip_sb[:], MUL
                        )
                        if h == 0 and pending[0] is not None:
                            emit_outproj(*pending[0])
                            pending[0] = None

                    pending[0] = (gi, osb)
                emit_outproj(*pending[0])

    nc.finalize()
    return nc


def _get_built():
    global _built
    if _built is None:
        _built = build_bass()
    return _built


def make_in_maps(hidden_states, Wq, Wk, Wv, Wo):
    cos_t, sin_t = rope_tables()
    ident = np.eye(P, dtype=np.float32)
    ones_mat = np.ones((P, P), np.float32)
    in_maps = []
    for core in range(N_CORES):
        b, g = divmod(core, NKV)
        in_maps.append(
            {
                "xT": np.ascontiguousarray(hidden_states[b].T),
                "wq": np.ascontiguousarray(Wq[:, g * GROUPS * HD : (g + 1) * GROUPS * HD]),
                "wk": np.ascontiguousarray(Wk[:, g * HD : (g + 1) * HD]),
                "wv": np.ascontiguousarray(Wv[:, g * HD : (g + 1) * HD]),
                "wo": np.ascontiguousarray(Wo[g * GROUPS * HD : (g + 1) * GROUPS * HD, :]),
                "cos_t": cos_t,
                "sin_t": sin_t,
                "ident": ident,
                "ones_mat": ones_mat,
            }
        )
    return in_maps


def kernel(hidden_states, Wq, Wk, Wv, Wo, trace=False):
    from concourse.bass_utils import run_bass_kernel_spmd

    hidden_states = np.asarray(hidden_states, dtype=np.float32)
    Wq = np.asarray(Wq, dtype=np.float32)
    Wk = np.asarray(Wk, dtype=np.float32)
    Wv = np.asarray(Wv, dtype=np.float32)
    Wo = np.asarray(Wo, dtype=np.float32)

    nc = _get_built()
    in_maps = make_in_maps(hidden_states, Wq, Wk, Wv, Wo)
    res = run_bass_kernel_spmd(nc, in_maps, core_ids=list(range(N_CORES)), trace=trace)

    out = np.zeros((B, S, HID), dtype=np.float32)
    for core in range(N_CORES):
        b = core // NKV
        out[b] += res.results[core]["out"]
    if trace:
        kernel.last_result = res
    return out


# revision 31
# speedup vs baseline: 1.0310x; 1.0310x over previous
"""Trainium2 Bass kernel for GQA attention (B=2, S=2048, HID=2048, 16 q-heads,
4 kv-heads, HD=128, RoPE, softmax, output projection).

Sharding: 8 cores = (2 batches) x (4 query-head groups of 4 heads). Each core
owns one batch's hidden states, 4 query heads, and the single kv head those
query heads attend to (GQA group), plus the matching 512-row slice of Wo.
Core (b, g) computes a [S, HID] partial of the output projection; the host
sums the 4 partials per batch (row-scaled softmax commutes with the
projection, so normalization happens on-device per head).

Dataflow per core (all layouts transposed so the contraction dim sits on
SBUF partitions; no DMA transposes needed for fp32):
  x^T via PE transposes -> Q^T/K^T/V^T projections (fp32r matmuls)
  -> RoPE on Q^T/K^T (DVE, sign-folded sin table)
  -> scores^T tiles = K^T_chunk.T @ Q^T (fp32r) -> exp on ACT (bf16 E tiles)
  -> softmax denominators via ones-vector matmul (PE), reciprocal (DVE),
     rank-1 ones x recip broadcast matmul (PE)
  -> attn@V with PE-transposed V chunks (bf16) -> normalize (DVE)
  -> output projection vs Wo slice (fp32r) -> partial [S, HID] to HBM.
"""

import sys
import types

sys.path.insert(0, "/opt/trn_rl_repo")

import numpy as np

B, S, HID = 2, 2048, 2048
NH, NKV, HD = 16, 4, 128
GROUPS = NH // NKV          # q heads per kv head == heads per core
ROPE_THETA = 10000.0
P = 128                     # SBUF partitions
SB = 512                    # s-block (matmul moving dim / psum bank)
N_CORES = 8

_built = None


def _install_ntff_hook():
    """antenv.axon_hooks is missing from the agent image, which silently
    disables trace=True; recreate it and register the ctypes NTFF hook."""
    if "antenv.axon_hooks" in sys.modules:
        return
    m = types.ModuleType("antenv.axon_hooks")
    m._hook = None
    m.set_axon_ntff_profile_hook = lambda h: setattr(m, "_hook", h)
    m.get_axon_ntff_profile_hook = lambda: m._hook
    sys.modules["antenv.axon_hooks"] = m
    try:
        import antenv

        antenv.axon_hooks = m
    except ImportError:
        pass
    try:
        sys.path.insert(0, "/root/.axon_site/trn_agent_boot")
        from trn_boot import _ntff_profile_via_ctypes

        hook = _ntff_profile_via_ctypes("/opt/axon/libaxon_pjrt.so")
        if hook is not None:
            m.set_axon_ntff_profile_hook(hook)
    except Exception:
        pass


_install_ntff_hook()


def rope_tables():
    """cos table and sign-folded sin table in [HD, S] (transposed) layout.

    sin_signed[d] = -sin for d < HD/2, +sin for d >= HD/2, so RoPE becomes
    out = q * cos + shifted(q) * sin_signed with shifted(q) a partition-half
    swap.
    """
    half = HD // 2
    inv_freq = 1.0 / (ROPE_THETA ** (np.arange(0, HD, 2, dtype=np.float64) / HD))
    t = np.arange(S, dtype=np.float64)
    freqs = np.outer(t, inv_freq)                      # [S, 64]
    emb = np.concatenate([freqs, freqs], axis=-1)      # [S, 128]
    cos_t = np.cos(emb).T.astype(np.float32).copy()    # [128, S]
    sin = np.sin(emb).T.astype(np.float32)
    sin_signed = sin.copy()
    sin_signed[:half] *= -1.0
    return cos_t, np.ascontiguousarray(sin_signed)


def build_bass(s=S, hid=HID):
    import concourse.mybir as mybir
    from concourse import bacc
    from concourse.tile import TileContext

    F32 = mybir.dt.float32
    F32R = mybir.dt.float32r
    BF16 = mybir.dt.bfloat16
    EXP = mybir.ActivationFunctionType.Exp
    MUL = mybir.AluOpType.mult

    kc_n = hid // P            # hid contraction chunks
    sb_n = s // SB             # s blocks
    jc_n = s // P              # key chunks
    nh = GROUPS                # heads on this core
    scale = 1.0 / float(np.sqrt(HD))

    nc = bacc.Bacc("TRN2")

    xT_d = nc.dram_tensor("xT", [hid, s], F32R, kind="ExternalInput")
    wq = nc.dram_tensor("wq", [hid, nh * HD], F32R, kind="ExternalInput")
    wk = nc.dram_tensor("wk", [hid, HD], F32R, kind="ExternalInput")
    wv = nc.dram_tensor("wv", [hid, HD], F32R, kind="ExternalInput")
    wo = nc.dram_tensor("wo", [nh * HD, hid], F32, kind="ExternalInput")
    cos_d = nc.dram_tensor("cos_t", [P, s], F32, kind="ExternalInput")
    sin_d = nc.dram_tensor("sin_t", [P, s], F32, kind="ExternalInput")
    ident_d = nc.dram_tensor("ident", [P, P], F32, kind="ExternalInput")
    ones_mat_d = nc.dram_tensor("ones_mat", [P, P], F32, kind="ExternalInput")
    out = nc.dram_tensor("out", [s, hid], F32, kind="ExternalOutput")

    xT_r = xT_d.rearrange("(ko ki) s -> ki ko s", ki=P)
    wq_r = wq.rearrange("(ko ki) m -> ki ko m", ki=P)
    wk_r = wk.rearrange("(ko ki) m -> ki ko m", ki=P)
    wv_r = wv.rearrange("(ko ki) m -> ki ko m", ki=P)
    wo_r = wo.rearrange("(h ki) o -> ki h o", ki=P)

    with TileContext(nc) as tc:
        with (
            tc.tile_pool(name="const", bufs=1) as cpool,
            tc.tile_pool(name="qkvt", bufs=1) as qkvt,
            tc.tile_pool(name="ropetmp", bufs=2) as rtmp,
            tc.tile_pool(name="vtmp", bufs=2) as vtmpp,
        ):
            ident = cpool.tile([P, P], F32, tag="ident")
            nc.scalar.dma_start(ident[:], ident_d[:, :])
            ones_mat = cpool.tile([P, P], F32, tag="onesm")
            nc.scalar.dma_start(ones_mat[:], ones_mat_d[:, :])
            ident_bf = cpool.tile([P, P], BF16, tag="identbf")
            nc.vector.tensor_copy(ident_bf[:], ident[:])
            ones_mat_bf = cpool.tile([P, P], BF16, tag="onesmbf")
            nc.vector.tensor_copy(ones_mat_bf[:], ones_mat[:])

            qT = qkvt.tile([P, nh, s], F32R, tag="qT")       # [d, h, s]
            kT = qkvt.tile([P, s], F32R, tag="kT")           # [d, s]
            vnat = qkvt.tile([P, jc_n, HD], BF16, tag="vn")  # [j, jc, d]

            def rope(dst, src_ps, gs):
                # dst = src * cos + shifted_halves(src) * sin_signed
                h2 = HD // 2
                tmp = rtmp.tile([P, SB], F32, tag="ropetmp")
                nc.vector.tensor_tensor(dst, src_ps[:], cos_t[:, gs : gs + SB], MUL)
                nc.vector.tensor_tensor(
                    tmp[0:h2, :], src_ps[h2:P, :], sin_t[0:h2, gs : gs + SB], MUL
                )
                nc.vector.tensor_tensor(
                    tmp[h2:P, :], src_ps[0:h2, :], sin_t[h2:P, gs : gs + SB], MUL
                )
                nc.vector.tensor_add(dst, dst, tmp[:])

            # ---- Phase 1: x^T, projections, RoPE, V transpose (per s-block) --
            with (
                tc.tile_pool(name="xt", bufs=2) as xtp,
                tc.tile_pool(name="wsb", bufs=1) as wsb,
                tc.tile_pool(name="psA", bufs=2, space="PSUM") as psA,
                tc.tile_pool(name="psB", bufs=2, space="PSUM") as psB,
            ):
                wq_sb = wsb.tile([P, kc_n, nh * HD], F32R, tag="wq")
                wk_sb = wsb.tile([P, kc_n, HD], F32R, tag="wk")
                wv_sb = wsb.tile([P, kc_n, HD], F32R, tag="wv")
                kq = max(1, kc_n // 8)
                for c0 in range(0, kc_n, kq):
                    c1 = min(c0 + kq, kc_n)
                    nc.scalar.dma_start(wq_sb[:, c0:c1, :], wq_r[:, c0:c1, :])
                nc.scalar.dma_start(wk_sb[:], wk_r[:, :, :])
                nc.scalar.dma_start(wv_sb[:], wv_r[:, :, :])
                cos_t = cpool.tile([P, s], F32, tag="cos")
                sin_t = cpool.tile([P, s], F32, tag="sin")

                for sb in range(sb_n):
                    gs = sb * SB
                    xt = xtp.tile([P, kc_n, SB], F32R, tag="xt")
                    kq = max(1, kc_n // 8)
                    for ci, c0 in enumerate(range(0, kc_n, kq)):
                        c1 = min(c0 + kq, kc_n)
                        eng = nc.sync
                        eng.dma_start(
                            xt[:, c0:c1, :], xT_r[:, c0:c1, gs : gs + SB]
                        )
                    nc.sync.dma_start(cos_t[:, gs : gs + SB], cos_d[:, gs : gs + SB])
                    nc.sync.dma_start(sin_t[:, gs : gs + SB], sin_d[:, gs : gs + SB])

                    for h in range(nh):
                        q_ps = psB.tile([P, SB], F32, tag="proj")
                        for kc in range(kc_n):
                            nc.tensor.matmul(
                                q_ps[:],
                                wq_sb[:, kc, h * HD : (h + 1) * HD],
                                xt[:, kc, :],
                                start=(kc == 0),
                                stop=(kc == kc_n - 1),
                            )
                        rope(qT[:, h, gs : gs + SB], q_ps, gs)

                    k_ps = psB.tile([P, SB], F32, tag="proj")
                    for kc in range(kc_n):
                        nc.tensor.matmul(
                            k_ps[:],
                            wk_sb[:, kc, :],
                            xt[:, kc, :],
                            start=(kc == 0),
                            stop=(kc == kc_n - 1),
                        )
                    rope(kT[:, gs : gs + SB], k_ps, gs)

                    v_ps = psB.tile([P, SB], F32, tag="proj")
                    for kc in range(kc_n):
                        nc.tensor.matmul(
                            v_ps[:],
                            wv_sb[:, kc, :],
                            xt[:, kc, :],
                            start=(kc == 0),
                            stop=(kc == kc_n - 1),
                        )
                    vtmp = vtmpp.tile([P, SB], BF16, tag="vtmp")
                    nc.scalar.copy(vtmp[:], v_ps[:])
                    tps2 = psA.tile([P, SB], BF16, tag="tpsbf")
                    for t in range(SB // P):
                        nc.tensor.transpose(
                            tps2[:, t * P : (t + 1) * P],
                            vtmp[:, t * P : (t + 1) * P],
                            ident_bf[:],
                        )
                    jc0 = gs // P
                    nc.scalar.copy(
                        vnat[:, jc0 : jc0 + SB // P, :],
                        tps2[:].rearrange("p (a b) -> p a b", a=SB // P),
                    )

            # ---- Phase 2: attention + output projection (per i-block) -------
            with (
                tc.tile_pool(name="wo", bufs=1) as wop,
                tc.tile_pool(name="epool", bufs=2) as epool,
                tc.tile_pool(name="osb", bufs=2) as osbp,
                tc.tile_pool(name="recip", bufs=2) as rpool,
                tc.tile_pool(name="outsb", bufs=3) as outp,
                tc.tile_pool(name="psS", bufs=2, space="PSUM") as psS,
                tc.tile_pool(name="psP", bufs=2, space="PSUM") as psP,
                tc.tile_pool(name="psDen", bufs=1, space="PSUM") as psDen,
                tc.tile_pool(name="psO", bufs=1, space="PSUM") as psO,
            ):
                wo_sb = wop.tile([P, nh, hid], F32, tag="wo")
                nc.scalar.dma_start(wo_sb[:], wo_r[:, :, :])
                wo_bf = wop.tile([P, nh, hid], BF16, tag="wobf")
                for wh in range(nh):
                    nc.gpsimd.tensor_copy(wo_bf[:, wh, :], wo_sb[:, wh, :])

                def emit_outproj(po_gi, po_osb):
                    for po_ic in range(SB // P):
                        for po_oc in range(hid // SB):
                            po_ps = psP.tile([P, SB], F32, tag="projps")
                            for po_h in range(nh):
                                nc.tensor.matmul(
                                    po_ps[:],
                                    po_osb[:, po_h, po_ic * P : (po_ic + 1) * P],
                                    wo_bf[:, po_h, po_oc * SB : (po_oc + 1) * SB],
                                    start=(po_h == 0),
                                    stop=(po_h == nh - 1),
                                )
                            po_out = outp.tile([P, SB], F32, tag="outsb")
                            nc.vector.tensor_copy(po_out[:], po_ps[:])
                            nc.sync.dma_start(
                                out[po_gi + po_ic * P : po_gi + (po_ic + 1) * P,
                                    po_oc * SB : (po_oc + 1) * SB],
                                po_out[:],
                            )

                pending = [None]
                for ib in range(sb_n):
                    gi = ib * SB
                    osb = osbp.tile([P, nh, SB], BF16, tag="osb")
                    for h in range(nh):
                        e_t = epool.tile([P, jc_n, SB], BF16, tag="E")
                        den_ps = psDen.tile([P, SB], F32, tag="den")
                        o_ps = psO.tile([P, SB], F32, tag="opsum")
                        esum = rpool.tile([P, SB], BF16, tag="esum")
                        for jp in range(jc_n // 2):
                            jc0, jc1 = 2 * jp, 2 * jp + 1
                            s_ps = psS.tile([P, 2, SB], F32, tag="spsum")
                            nc.tensor.matmul(
                                s_ps[:, 0, :],
                                kT[:, jc0 * P : (jc0 + 1) * P],
                                qT[:, h, gi : gi + SB],
                                start=True,
                                stop=True,
                            )
                            nc.tensor.matmul(
                                s_ps[:, 1, :],
                                kT[:, jc1 * P : (jc1 + 1) * P],
                                qT[:, h, gi : gi + SB],
                                start=True,
                                stop=True,
                            )
                            nc.scalar.activation(
                                e_t[:, jc0 : jc0 + 2, :], s_ps[:], EXP, scale=scale
                            )
                            if jp == 0:
                                nc.vector.tensor_add(
                                    esum[:], e_t[:, 0, :], e_t[:, 1, :]
                                )
                            else:
                                nc.vector.tensor_add(
                                    esum[:], esum[:], e_t[:, jc0, :]
                                )
                                nc.vector.tensor_add(
                                    esum[:], esum[:], e_t[:, jc1, :]
                                )
                            nc.tensor.matmul(
                                o_ps[:],
                                vnat[:, jc0, :],
                                e_t[:, jc0, :],
                                start=(jc0 == 0),
                                stop=False,
                            )
                            nc.tensor.matmul(
                                o_ps[:],
                                vnat[:, jc1, :],
                                e_t[:, jc1, :],
                                start=False,
                                stop=(jc1 == jc_n - 1),
                            )
                        nc.tensor.matmul(
                            den_ps[:],
                            ones_mat_bf[:],
                            esum[:],
                            start=True,
                            stop=True,
                        )
                        recip_sb = rpool.tile([P, SB], F32, tag="recipsb")
                        scratch = rpool.tile([P, SB], F32, tag="rscratch")
                        nc.vector.reciprocal_approx_accurate(
                            recip_sb[:], den_ps[:], scratch[:]
                        )
                        nc.vector.tensor_tensor(
                            osb[:, h, :], o_ps[:], recip_sb[:], MUL
                        )
                        if h == 0 and pending[0] is not None:
                            emit_outproj(*pending[0])
                            pending[0] = None

                    pending[0] = (gi, osb)
                emit_outproj(*pending[0])

    nc.finalize()
    return nc


def _get_built():
    global _built
    if _built is None:
        _built = build_bass()
    return _built


def make_in_maps(hidden_states, Wq, Wk, Wv, Wo):
    cos_t, sin_t = rope_tables()
    ident = np.eye(P, dtype=np.float32)
    ones_mat = np.ones((P, P), np.float32)
    in_maps = []
    for core in range(N_CORES):
        b, g = divmod(core, NKV)
        in_maps.append(
            {
                "xT": np.ascontiguousarray(hidden_states[b].T),
                "wq": np.ascontiguousarray(Wq[:, g * GROUPS * HD : (g + 1) * GROUPS * HD]),
                "wk": np.ascontiguousarray(Wk[:, g * HD : (g + 1) * HD]),
                "wv": np.ascontiguousarray(Wv[:, g * HD : (g + 1) * HD]),
                "wo": np.ascontiguousarray(Wo[g * GROUPS * HD : (g + 1) * GROUPS * HD, :]),
                "cos_t": cos_t,
                "sin_t": sin_t,
                "ident": ident,
                "ones_mat": ones_mat,
            }
        )
    return in_maps


def kernel(hidden_states, Wq, Wk, Wv, Wo, trace=False):
    from concourse.bass_utils import run_bass_kernel_spmd

    hidden_states = np.asarray(hidden_states, dtype=np.float32)
    Wq = np.asarray(Wq, dtype=np.float32)
    Wk = np.asarray(Wk, dtype=np.float32)
    Wv = np.asarray(Wv, dtype=np.float32)
    Wo = np.asarray(Wo, dtype=np.float32)

    nc = _get_built()
    in_maps = make_in_maps(hidden_states, Wq, Wk, Wv, Wo)
    res = run_bass_kernel_spmd(nc, in_maps, core_ids=list(range(N_CORES)), trace=trace)

    out = np.zeros((B, S, HID), dtype=np.float32)
    for core in range(N_CORES):
        b = core // NKV
        out[b] += res.results[core]["out"]
    if trace:
        kernel.last_result = res
    return out
